# revision 17
# baseline (speedup 1.0000x reference)
"""Conv2D 3x3 stride-1 pad-1 (NCHW) as implicit GEMM on 8 NeuronCores.

Strategy: data-parallel over batch (32 imgs -> 4 per core). The input is
zero-padded on the host to (*, 128, 58, 58), converted to bf16, and all
4 images DMA into one resident SBUF tile [C=128, 4, 58, 58] (27KB per
partition) so no load ever waits on compute. Weights are preprocessed
host-side to bf16 [I=128, (kh kw o)] so each (tap, ochunk) slice is a
ready [K=128, M=128] stationary operand; bf16 LDWEIGHTS (91ns) hides
fully under the previous 448-col matmul (187ns), unlike fp32r whose
4-byte weight load serializes ~23ns per matmul.

Loop order: image -> row-group (8 rows, free dim 448) -> ochunk -> tap.
Taps innermost means each PSUM group completes every ~1.8us and its
bias-add + output DMA overlap the next group's matmuls -- the drain
stream spreads across the whole kernel instead of piling into a tail.
Output DMAs alternate between the two hardware DGE queues (sync, scalar);
images 2-3 load on the gpsimd software queue, an independent rail.

x (4,128,58,58) bf16 -> out (4,256,56,56) f32 per core; no collectives.
"""

import os
import sys

import numpy as np

if "/opt/trn_rl_repo" not in sys.path:
    sys.path.insert(0, "/opt/trn_rl_repo")

from concourse import bacc, bass, mybir  # noqa: E402
from concourse.bass_utils import run_bass_kernel_spmd  # noqa: E402
from concourse.tile import TileContext, add_dep_helper  # noqa: E402

N_FULL, CIN, H, W = 32, 128, 56, 56
COUT = 256
KH = KW = 3
NCORES = 8
NPER = N_FULL // NCORES  # 4 images per core
HP, WP = H + 2, W + 2  # 58 x 58 padded
ROWS = 8  # output rows per matmul group
NFREE = ROWS * W  # 448 moving free dim
NGROUPS = H // ROWS  # 7
OCH = COUT // 128  # 2 output-channel chunks

_CACHE = {}


def _build_conv():
    f32 = mybir.dt.float32
    bf16 = mybir.dt.bfloat16

    # Bacc (not raw Bass): its compile pipeline legalizes sync waits --
    # TRN2 instructions carry at most one wait slot.
    nc = bacc.Bacc(None, target_bir_lowering=False)

    x_par = nc.declare_dram_parameter("x", [NPER, CIN, HP, WP], bf16, isOutput=False)
    w_par = nc.declare_dram_parameter(
        "wt", [CIN, KH * KW * COUT], bf16, isOutput=False
    )
    # bias comes in host-pretransposed as [128, OCH] so the DMA is a
    # contiguous 8B-per-partition transfer instead of a 256-packet scatter.
    bias_par = nc.declare_dram_parameter("bias", [128, OCH], f32, isOutput=False)
    out_par = nc.declare_dram_parameter("out", [NPER, COUT, H, W], f32, isOutput=True)
    out_flat = out_par.rearrange("n o h w -> n o (h w)")

    with TileContext(nc) as tc:
        with (
            tc.tile_pool(name="const", bufs=1) as cpool,
            tc.tile_pool(name="psum", bufs=6, space="PSUM") as ppool,
            tc.tile_pool(name="outp", bufs=6) as opool,
        ):
            # HAM pre-warm: junk matmuls gated only on a prologue memset run
            # during the initial DMA wait so the PE clock gate is at 8/8
            # (2.4 GHz) when the real stream starts. Results never consumed.
            jnk = cpool.tile([128, 512], f32, tag="jnk")
            # gpsimd clears its framework prologue ~1.5us before vector, so
            # gating the warm-up on a gpsimd memset starts it that much
            # earlier.
            nc.gpsimd.memset(jnk[:], 1.0)
            jnk_mm = jnk.bitcast(bf16)
            ps_jnk = ppool.tile([128, NFREE], f32, tag="ps", name="ps")
            for _ in range(7):
                nc.tensor.matmul(
                    ps_jnk[:],
                    jnk_mm[:, 0:128],
                    jnk_mm[:, 0:NFREE],
                    start=True,
                    stop=True,
                )

            # All four images resident: one [C, n, h, w] tile, 27KB/partition.
            x_sb = cpool.tile([CIN, NPER, HP, WP], bf16, tag="xall", name="xall")
            w_sb = cpool.tile([CIN, KH * KW * COUT], bf16, tag="w", name="w")
            bias_sb = cpool.tile([128, OCH], f32, tag="bias")

            w3_sb = w_sb.rearrange("p (t o) -> p t o", t=KH * KW)
            w3_dr = w_par[:].rearrange("p (t o) -> p t o", t=KH * KW)

            # Head loads. The PE consumes ALL 18 weight slices within the
            # first two row-groups (~3.2us of matmuls), so w is the
            # latency-critical tensor: split it across both HW rings ahead
            # of everything except the first x rows. Images 2-3 (the bulk)
            # are deferred behind the first real matmul so they don't steal
            # DMA-engine bandwidth from the head; they still land ~30us
            # before their deadlines.
            # Three rails: scalar ring carries w taps 0-4 (one DMA, big
            # packets), sync ring carries image-0 row chunks in consumption
            # order, and the gpsimd software ring carries w taps 5-8 (its
            # only head-critical item) followed by images 1-3, which are
            # deferred behind the first real matmul so the bulk never
            # contends with the head.
            nc.scalar.dma_start(out=w3_sb[:, 0:3, :], in_=w3_dr[:, 0:3, :])
            nc.sync.dma_start(out=x_sb[:, 0, 0:10, :], in_=x_par[0, :, 0:10, :])
            nc.scalar.dma_start(out=w3_sb[:, 3:5, :], in_=w3_dr[:, 3:5, :])
            nc.gpsimd.dma_start(out=w3_sb[:, 5:9, :], in_=w3_dr[:, 5:9, :])
            nc.scalar.dma_start(out=bias_sb[:], in_=bias_par[:])
            nc.sync.dma_start(out=x_sb[:, 0, 10:26, :], in_=x_par[0, :, 10:26, :])
            nc.sync.dma_start(out=x_sb[:, 0, 26:42, :], in_=x_par[0, :, 26:42, :])
            nc.sync.dma_start(out=x_sb[:, 0, 42:58, :], in_=x_par[0, :, 42:58, :])
            img_dmas = [
                nc.gpsimd.dma_start(out=x_sb[:, 1, :, :], in_=x_par[1]),
                nc.gpsimd.dma_start(out=x_sb[:, 2, :, :], in_=x_par[2]),
                nc.gpsimd.dma_start(out=x_sb[:, 3, :, :], in_=x_par[3]),
            ]

            mm_first = None

            # Row groups per image: 7x8 rows, except the last image ends with
            # a 6+2 split so the final accumulate->drain->store chain after
            # the very last matmul is ~4x shorter.
            full_groups = [(g * ROWS, ROWS) for g in range(NGROUPS)]
            tail_groups = full_groups[:-1] + [(48, 6), (54, 2)]

            drain_idx = 0
            for n in range(NPER):
                groups = tail_groups if n == NPER - 1 else full_groups
                for gi, (r0, nrows) in enumerate(groups):
                    nfree = nrows * W
                    for oc in range(OCH):
                        ps = ppool.tile([128, nfree], f32, tag="ps", name="ps")
                        for tap in range(KH * KW):
                            kh, kw = divmod(tap, KW)
                            mm = nc.tensor.matmul(
                                ps[:],
                                w3_sb[:, tap, oc * 128 : oc * 128 + 128],
                                x_sb[:, n, r0 + kh : r0 + kh + nrows, kw : kw + W],
                                start=(tap == 0),
                                stop=(tap == KH * KW - 1),
                            )
                            if mm_first is None:
                                mm_first = mm
                        ot = opool.tile([128, nfree], f32, tag="ot", name="ot")
                        nc.vector.tensor_scalar_add(
                            out=ot[:], in0=ps[:], scalar1=bias_sb[:, oc : oc + 1]
                        )
                        dst = out_flat[
                            n,
                            oc * 128 : (oc + 1) * 128,
                            r0 * W : r0 * W + nfree,
                        ]
                        last = (
                            n == NPER - 1
                            and gi == len(groups) - 1
                            and oc == OCH - 1
                        )
                        if last:
                            # Tail latency: split the final store across both
                            # HW queues.
                            half = nfree // 2
                            nc.sync.dma_start(out=dst[:, 0:half], in_=ot[:, 0:half])
                            nc.scalar.dma_start(
                                out=dst[:, half:nfree], in_=ot[:, half:nfree]
                            )
                        elif drain_idx % 2 == 0:
                            nc.sync.dma_start(out=dst, in_=ot[:])
                        else:
                            nc.scalar.dma_start(out=dst, in_=ot[:])
                        drain_idx += 1
            for dma in img_dmas:
                add_dep_helper(
                    dma.ins,
                    mm_first.ins,
                    sync=True,
                    reason="defer bulk image loads past the head",
                )
    nc.compile()
    return nc


def _get_nc():
    if "nc" not in _CACHE:
        _CACHE["nc"] = _build_conv()
    return _CACHE["nc"]


# test-harness hooks: set TRACE=True before calling kernel() to capture an
# NTFF profile; LAST_RESULTS then holds the BassKernelResults.
TRACE = False
LAST_RESULTS = None


def kernel(x, weight, bias):
    global LAST_RESULTS
    import ml_dtypes

    bfl = ml_dtypes.bfloat16
    x = np.ascontiguousarray(np.asarray(x), dtype=np.float32)
    w = np.ascontiguousarray(np.asarray(weight), dtype=np.float32)
    b = np.ascontiguousarray(np.asarray(bias), dtype=np.float32)
    xp = np.pad(x, ((0, 0), (0, 0), (1, 1), (1, 1))).astype(bfl)
    # wt[i, (kh kw o)] = w[o, i, kh, kw]
    wt = np.ascontiguousarray(
        w.transpose(1, 2, 3, 0).reshape(CIN, KH * KW * COUT)
    ).astype(bfl)

    b2 = np.ascontiguousarray(b.reshape(OCH, 128).T)  # [128, OCH]
    per_core = [
        {"x": xp[c * NPER : (c + 1) * NPER], "wt": wt, "bias": b2}
        for c in range(NCORES)
    ]

    kwargs = {}
    if TRACE:
        kwargs = dict(trace=True, trace_cores=[0])
    res = run_bass_kernel_spmd(
        _get_nc(), per_core, core_ids=list(range(NCORES)), **kwargs
    )
    LAST_RESULTS = res
    return np.concatenate([r["out"] for r in res.results], axis=0)


# revision 22
# speedup vs baseline: 1.0011x; 1.0011x over previous
"""Conv2D 3x3 stride-1 pad-1 (NCHW) as implicit GEMM on 8 NeuronCores.

Strategy: data-parallel over batch (32 imgs -> 4 per core). The input is
zero-padded on the host to (*, 128, 58, 58), converted to bf16, and all
4 images DMA into one resident SBUF tile [C=128, 4, 58, 58] (27KB per
partition) so no load ever waits on compute. Weights are preprocessed
host-side to bf16 [I=128, (kh kw o)] so each (tap, ochunk) slice is a
ready [K=128, M=128] stationary operand; bf16 LDWEIGHTS (91ns) hides
fully under the previous 448-col matmul (187ns), unlike fp32r whose
4-byte weight load serializes ~23ns per matmul.

Loop order: image -> row-group (8 rows, free dim 448) -> ochunk -> tap.
Taps innermost means each PSUM group completes every ~1.8us and its
bias-add + output DMA overlap the next group's matmuls -- the drain
stream spreads across the whole kernel instead of piling into a tail.
Output DMAs alternate between the two hardware DGE queues (sync, scalar);
images 2-3 load on the gpsimd software queue, an independent rail.

x (4,128,58,58) bf16 -> out (4,256,56,56) f32 per core; no collectives.
"""

import os
import sys

import numpy as np

if "/opt/trn_rl_repo" not in sys.path:
    sys.path.insert(0, "/opt/trn_rl_repo")

from concourse import bacc, bass, mybir  # noqa: E402
from concourse.bass_utils import run_bass_kernel_spmd  # noqa: E402
from concourse.tile import TileContext, add_dep_helper  # noqa: E402

N_FULL, CIN, H, W = 32, 128, 56, 56
COUT = 256
KH = KW = 3
NCORES = 8
NPER = N_FULL // NCORES  # 4 images per core
HP, WP = H + 2, W + 2  # 58 x 58 padded
ROWS = 8  # output rows per matmul group
NFREE = ROWS * W  # 448 moving free dim
NGROUPS = H // ROWS  # 7
OCH = COUT // 128  # 2 output-channel chunks

_CACHE = {}


def _build_conv():
    f32 = mybir.dt.float32
    bf16 = mybir.dt.bfloat16

    # Bacc (not raw Bass): its compile pipeline legalizes sync waits --
    # TRN2 instructions carry at most one wait slot.
    nc = bacc.Bacc(None, target_bir_lowering=False)

    x_par = nc.declare_dram_parameter("x", [NPER, CIN, HP, WP], bf16, isOutput=False)
    # oc-major weight layout: each oc-half is one contiguous 2304B/partition
    # block, loaded as the FIRST DMA of its ring (sem lag grows with ring
    # depth, so the head-critical tensors must lead their rings).
    w_par = nc.declare_dram_parameter(
        "wt", [CIN, OCH, KH * KW, 128], bf16, isOutput=False
    )
    # bias comes in host-pretransposed as [128, OCH] so the DMA is a
    # contiguous 8B-per-partition transfer instead of a 256-packet scatter.
    bias_par = nc.declare_dram_parameter("bias", [128, OCH], f32, isOutput=False)
    out_par = nc.declare_dram_parameter("out", [NPER, COUT, H, W], f32, isOutput=True)
    out_flat = out_par.rearrange("n o h w -> n o (h w)")

    with TileContext(nc) as tc:
        with (
            tc.tile_pool(name="const", bufs=1) as cpool,
            tc.tile_pool(name="psum", bufs=6, space="PSUM") as ppool,
            tc.tile_pool(name="outp", bufs=6) as opool,
        ):
            # HAM pre-warm: junk matmuls gated only on a prologue memset run
            # during the initial DMA wait so the PE clock gate is at 8/8
            # (2.4 GHz) when the real stream starts. Results never consumed.
            jnk = cpool.tile([128, 512], f32, tag="jnk")
            # gpsimd clears its framework prologue ~1.5us before vector, so
            # gating the warm-up on a gpsimd memset starts it that much
            # earlier.
            nc.gpsimd.memset(jnk[:], 1.0)
            jnk_mm = jnk.bitcast(bf16)
            ps_jnk = ppool.tile([128, NFREE], f32, tag="ps", name="ps")
            for _ in range(7):
                nc.tensor.matmul(
                    ps_jnk[:],
                    jnk_mm[:, 0:128],
                    jnk_mm[:, 0:NFREE],
                    start=True,
                    stop=True,
                )

            # All four images resident: one [C, n, h, w] tile, 27KB/partition.
            x_sb = cpool.tile([CIN, NPER, HP, WP], bf16, tag="xall", name="xall")
            w3_sb = cpool.tile([CIN, OCH, KH * KW, 128], bf16, tag="w", name="w")
            bias_sb = cpool.tile([128, OCH], f32, tag="bias")

            # Head loads. The PE consumes ALL 18 weight slices within the
            # first two row-groups (~3.2us of matmuls), so w is the
            # latency-critical tensor: split it across both HW rings ahead
            # of everything except the first x rows. Images 2-3 (the bulk)
            # are deferred behind the first real matmul so they don't steal
            # DMA-engine bandwidth from the head; they still land ~30us
            # before their deadlines.
            # Three rails, each led by its head-critical tensor: scalar ring
            # leads with w-oc0, the gpsimd software ring leads with w-oc1
            # (needed ~1.7us later) before the bulk images, and the sync ring
            # leads with image-0 row chunks in consumption order. Images 1-3
            # are deferred behind the first real matmul so they never
            # contend with the head.
            nc.scalar.dma_start(out=w3_sb[:, 0], in_=w_par[:, 0])
            nc.sync.dma_start(out=x_sb[:, 0, 0:10, :], in_=x_par[0, :, 0:10, :])
            nc.gpsimd.dma_start(out=w3_sb[:, 1], in_=w_par[:, 1])
            nc.scalar.dma_start(out=bias_sb[:], in_=bias_par[:])
            nc.sync.dma_start(out=x_sb[:, 0, 10:26, :], in_=x_par[0, :, 10:26, :])
            nc.sync.dma_start(out=x_sb[:, 0, 26:42, :], in_=x_par[0, :, 26:42, :])
            nc.sync.dma_start(out=x_sb[:, 0, 42:58, :], in_=x_par[0, :, 42:58, :])
            img_dmas = [
                nc.gpsimd.dma_start(out=x_sb[:, 1, :, :], in_=x_par[1]),
                nc.gpsimd.dma_start(out=x_sb[:, 2, :, :], in_=x_par[2]),
                nc.gpsimd.dma_start(out=x_sb[:, 3, :, :], in_=x_par[3]),
            ]

            mm_first = None

            # Row groups per image: 7x8 rows, except the last image ends with
            # a 6+2 split so the final accumulate->drain->store chain after
            # the very last matmul is ~4x shorter.
            full_groups = [(g * ROWS, ROWS) for g in range(NGROUPS)]
            tail_groups = full_groups[:-1] + [(48, 6), (54, 2)]

            drain_idx = 0
            for n in range(NPER):
                groups = tail_groups if n == NPER - 1 else full_groups
                for gi, (r0, nrows) in enumerate(groups):
                    nfree = nrows * W
                    for oc in range(OCH):
                        ps = ppool.tile([128, nfree], f32, tag="ps", name="ps")
                        for tap in range(KH * KW):
                            kh, kw = divmod(tap, KW)
                            mm = nc.tensor.matmul(
                                ps[:],
                                w3_sb[:, oc, tap, :],
                                x_sb[:, n, r0 + kh : r0 + kh + nrows, kw : kw + W],
                                start=(tap == 0),
                                stop=(tap == KH * KW - 1),
                            )
                            if mm_first is None:
                                mm_first = mm
                        ot = opool.tile([128, nfree], f32, tag="ot", name="ot")
                        nc.vector.tensor_scalar_add(
                            out=ot[:], in0=ps[:], scalar1=bias_sb[:, oc : oc + 1]
                        )
                        dst = out_flat[
                            n,
                            oc * 128 : (oc + 1) * 128,
                            r0 * W : r0 * W + nfree,
                        ]
                        last = (
                            n == NPER - 1
                            and gi == len(groups) - 1
                            and oc == OCH - 1
                        )
                        if last:
                            # Tail latency: split the final store across both
                            # HW queues.
                            half = nfree // 2
                            nc.sync.dma_start(out=dst[:, 0:half], in_=ot[:, 0:half])
                            nc.scalar.dma_start(
                                out=dst[:, half:nfree], in_=ot[:, half:nfree]
                            )
                        elif drain_idx % 2 == 0:
                            nc.sync.dma_start(out=dst, in_=ot[:])
                        else:
                            nc.scalar.dma_start(out=dst, in_=ot[:])
                        drain_idx += 1
            for dma in img_dmas:
                add_dep_helper(
                    dma.ins,
                    mm_first.ins,
                    sync=True,
                    reason="defer bulk image loads past the head",
                )
    nc.compile()
    return nc


def _get_nc():
    if "nc" not in _CACHE:
        _CACHE["nc"] = _build_conv()
    return _CACHE["nc"]


# test-harness hooks: set TRACE=True before calling kernel() to capture an
# NTFF profile; LAST_RESULTS then holds the BassKernelResults.
TRACE = False
LAST_RESULTS = None


def kernel(x, weight, bias):
    global LAST_RESULTS
    import ml_dtypes

    bfl = ml_dtypes.bfloat16
    x = np.ascontiguousarray(np.asarray(x), dtype=np.float32)
    w = np.ascontiguousarray(np.asarray(weight), dtype=np.float32)
    b = np.ascontiguousarray(np.asarray(bias), dtype=np.float32)
    xp = np.pad(x, ((0, 0), (0, 0), (1, 1), (1, 1))).astype(bfl)
    # wt[i, oc, (kh kw), j] = w[oc*128+j, i, kh, kw]  (oc-major halves)
    wt = np.ascontiguousarray(
        w.transpose(1, 2, 3, 0)
        .reshape(CIN, KH * KW, OCH, 128)
        .transpose(0, 2, 1, 3)
    ).astype(bfl)

    b2 = np.ascontiguousarray(b.reshape(OCH, 128).T)  # [128, OCH]
    per_core = [
        {"x": xp[c * NPER : (c + 1) * NPER], "wt": wt, "bias": b2}
        for c in range(NCORES)
    ]

    kwargs = {}
    if TRACE:
        kwargs = dict(trace=True, trace_cores=[0])
    res = run_bass_kernel_spmd(
        _get_nc(), per_core, core_ids=list(range(NCORES)), **kwargs
    )
    LAST_RESULTS = res
    return np.concatenate([r["out"] for r in res.results], axis=0)


# revision 28
# speedup vs baseline: 1.0260x; 1.0249x over previous
"""Conv2D 3x3 stride-1 pad-1 (NCHW) as implicit GEMM on 8 NeuronCores.

Strategy: data-parallel over batch (32 imgs -> 4 per core). The input is
zero-padded on the host to (*, 128, 58, 58), converted to bf16, and all
4 images DMA into one resident SBUF tile [C=128, 4, 58, 58] (27KB per
partition) so no load ever waits on compute. Weights are preprocessed
host-side to bf16 [I=128, (kh kw o)] so each (tap, ochunk) slice is a
ready [K=128, M=128] stationary operand; bf16 LDWEIGHTS (91ns) hides
fully under the previous 448-col matmul (187ns), unlike fp32r whose
4-byte weight load serializes ~23ns per matmul.

Loop order: image -> row-group (8 rows, free dim 448) -> ochunk -> tap.
Taps innermost means each PSUM group completes every ~1.8us and its
bias-add + output DMA overlap the next group's matmuls -- the drain
stream spreads across the whole kernel instead of piling into a tail.
Output DMAs alternate between the two hardware DGE queues (sync, scalar);
images 2-3 load on the gpsimd software queue, an independent rail.

x (4,128,58,58) bf16 -> out (4,256,56,56) f32 per core; no collectives.
"""

import os
import sys

import numpy as np

if "/opt/trn_rl_repo" not in sys.path:
    sys.path.insert(0, "/opt/trn_rl_repo")

from concourse import bacc, bass, mybir  # noqa: E402
from concourse.bass_utils import run_bass_kernel_spmd  # noqa: E402
from concourse.tile import TileContext, add_dep_helper  # noqa: E402

N_FULL, CIN, H, W = 32, 128, 56, 56
COUT = 256
KH = KW = 3
NCORES = 8
NPER = N_FULL // NCORES  # 4 images per core
HP, WP = H + 2, W + 2  # 58 x 58 padded
ROWS = 8  # output rows per matmul group
NFREE = ROWS * W  # 448 moving free dim
NGROUPS = H // ROWS  # 7
OCH = COUT // 128  # 2 output-channel chunks

_CACHE = {}


def _build_conv():
    f32 = mybir.dt.float32
    bf16 = mybir.dt.bfloat16

    # Bacc (not raw Bass): its compile pipeline legalizes sync waits --
    # TRN2 instructions carry at most one wait slot.
    nc = bacc.Bacc(None, target_bir_lowering=False)

    x_par = nc.declare_dram_parameter("x", [NPER, CIN, HP, WP], bf16, isOutput=False)
    w_par = nc.declare_dram_parameter(
        "wt", [CIN, KH * KW * COUT], bf16, isOutput=False
    )
    # bias comes in host-pretransposed as [128, OCH] so the DMA is a
    # contiguous 8B-per-partition transfer instead of a 256-packet scatter.
    bias_par = nc.declare_dram_parameter("bias", [128, OCH], f32, isOutput=False)
    out_par = nc.declare_dram_parameter("out", [NPER, COUT, H, W], f32, isOutput=True)
    out_flat = out_par.rearrange("n o h w -> n o (h w)")

    with TileContext(nc) as tc:
        with (
            tc.tile_pool(name="const", bufs=1) as cpool,
            tc.tile_pool(name="psum", bufs=6, space="PSUM") as ppool,
            tc.tile_pool(name="outp", bufs=6) as opool,
        ):
            # HAM pre-warm: junk matmuls gated only on a prologue memset run
            # during the initial DMA wait so the PE clock gate is at 8/8
            # (2.4 GHz) when the real stream starts. Results never consumed.
            jnk = cpool.tile([128, 512], f32, tag="jnk")
            # gpsimd clears its framework prologue ~1.5us before vector, so
            # gating the warm-up on a gpsimd memset starts it that much
            # earlier.
            nc.gpsimd.memset(jnk[:], 1.0)
            jnk_mm = jnk.bitcast(bf16)
            ps_jnk = ppool.tile([128, NFREE], f32, tag="ps", name="ps")
            for _ in range(8):
                nc.tensor.matmul(
                    ps_jnk[:],
                    jnk_mm[:, 0:128],
                    jnk_mm[:, 0:NFREE],
                    start=True,
                    stop=True,
                )

            # All four images resident: one [C, n, h, w] tile, 27KB/partition.
            x_sb = cpool.tile([CIN, NPER, HP, WP], bf16, tag="xall", name="xall")
            w_sb = cpool.tile([CIN, KH * KW * COUT], bf16, tag="w", name="w")
            bias_sb = cpool.tile([128, OCH], f32, tag="bias")

            w3_sb = w_sb.rearrange("p (t o) -> p t o", t=KH * KW)
            w3_dr = w_par[:].rearrange("p (t o) -> p t o", t=KH * KW)

            # Head loads. The PE consumes ALL 18 weight slices within the
            # first two row-groups (~3.2us of matmuls), so w is the
            # latency-critical tensor: split it across both HW rings ahead
            # of everything except the first x rows. Images 2-3 (the bulk)
            # are deferred behind the first real matmul so they don't steal
            # DMA-engine bandwidth from the head; they still land ~30us
            # before their deadlines.
            # Three rails, each led by its head-critical tensor: scalar ring
            # leads with w taps 0-4 (one DMA, big packets), the gpsimd
            # software ring leads with w taps 5-8 before the bulk images,
            # and the sync ring leads with image-0 row chunks in consumption
            # order. Images 1-3 are deferred behind the first real matmul so
            # they never contend with the head. Per-DMA completion
            # semaphores release ~1-2.5us after the data and the lag grows
            # with ring depth, so finer JIT chunking of w makes the head
            # slower, not faster.
            nc.scalar.dma_start(out=w3_sb[:, 0:5, :], in_=w3_dr[:, 0:5, :])
            nc.sync.dma_start(out=x_sb[:, 0, 0:10, :], in_=x_par[0, :, 0:10, :])
            nc.gpsimd.dma_start(out=w3_sb[:, 5:9, :], in_=w3_dr[:, 5:9, :])
            nc.scalar.dma_start(out=bias_sb[:], in_=bias_par[:])
            nc.sync.dma_start(out=x_sb[:, 0, 10:26, :], in_=x_par[0, :, 10:26, :])
            nc.sync.dma_start(out=x_sb[:, 0, 26:42, :], in_=x_par[0, :, 26:42, :])
            nc.sync.dma_start(out=x_sb[:, 0, 42:58, :], in_=x_par[0, :, 42:58, :])
            img_dmas = [
                nc.gpsimd.dma_start(out=x_sb[:, 1, :, :], in_=x_par[1]),
                nc.gpsimd.dma_start(out=x_sb[:, 2, :, :], in_=x_par[2]),
                nc.gpsimd.dma_start(out=x_sb[:, 3, :, :], in_=x_par[3]),
            ]

            mm_first = None

            # Row groups per image: 7x8 rows, except the last image ends with
            # a 6+2 split so the final accumulate->drain->store chain after
            # the very last matmul is ~4x shorter.
            full_groups = [(g * ROWS, ROWS) for g in range(NGROUPS)]
            tail_groups = full_groups[:-1] + [(48, 6), (54, 2)]

            drain_idx = 0
            for n in range(NPER):
                groups = tail_groups if n == NPER - 1 else full_groups
                for gi, (r0, nrows) in enumerate(groups):
                    nfree = nrows * W
                    for oc in range(OCH):
                        ps = ppool.tile([128, nfree], f32, tag="ps", name="ps")
                        for tap in range(KH * KW):
                            kh, kw = divmod(tap, KW)
                            mm = nc.tensor.matmul(
                                ps[:],
                                w3_sb[:, tap, oc * 128 : oc * 128 + 128],
                                x_sb[:, n, r0 + kh : r0 + kh + nrows, kw : kw + W],
                                start=(tap == 0),
                                stop=(tap == KH * KW - 1),
                            )
                            if mm_first is None:
                                mm_first = mm
                        ot = opool.tile([128, nfree], f32, tag="ot", name="ot")
                        nc.vector.tensor_scalar_add(
                            out=ot[:], in0=ps[:], scalar1=bias_sb[:, oc : oc + 1]
                        )
                        dst = out_flat[
                            n,
                            oc * 128 : (oc + 1) * 128,
                            r0 * W : r0 * W + nfree,
                        ]
                        last = (
                            n == NPER - 1
                            and gi == len(groups) - 1
                            and oc == OCH - 1
                        )
                        if last:
                            # Tail latency: split the final store across both
                            # HW queues.
                            half = nfree // 2
                            nc.sync.dma_start(out=dst[:, 0:half], in_=ot[:, 0:half])
                            nc.scalar.dma_start(
                                out=dst[:, half:nfree], in_=ot[:, half:nfree]
                            )
                        elif drain_idx % 2 == 0:
                            nc.sync.dma_start(out=dst, in_=ot[:])
                        else:
                            nc.scalar.dma_start(out=dst, in_=ot[:])
                        drain_idx += 1
            for dma in img_dmas:
                add_dep_helper(
                    dma.ins,
                    mm_first.ins,
                    sync=True,
                    reason="defer bulk image loads past the head",
                )
    nc.compile()
    return nc


def _get_nc():
    if "nc" not in _CACHE:
        _CACHE["nc"] = _build_conv()
    return _CACHE["nc"]


# test-harness hooks: set TRACE=True before calling kernel() to capture an
# NTFF profile; LAST_RESULTS then holds the BassKernelResults.
TRACE = False
LAST_RESULTS = None


def kernel(x, weight, bias):
    global LAST_RESULTS
    import ml_dtypes

    bfl = ml_dtypes.bfloat16
    x = np.ascontiguousarray(np.asarray(x), dtype=np.float32)
    w = np.ascontiguousarray(np.asarray(weight), dtype=np.float32)
    b = np.ascontiguousarray(np.asarray(bias), dtype=np.float32)
    xp = np.pad(x, ((0, 0), (0, 0), (1, 1), (1, 1))).astype(bfl)
    # wt[i, (kh kw o)] = w[o, i, kh, kw]
    wt = np.ascontiguousarray(
        w.transpose(1, 2, 3, 0).reshape(CIN, KH * KW * COUT)
    ).astype(bfl)

    b2 = np.ascontiguousarray(b.reshape(OCH, 128).T)  # [128, OCH]
    per_core = [
        {"x": xp[c * NPER : (c + 1) * NPER], "wt": wt, "bias": b2}
        for c in range(NCORES)
    ]

    kwargs = {}
    if TRACE:
        kwargs = dict(trace=True, trace_cores=[0])
    res = run_bass_kernel_spmd(
        _get_nc(), per_core, core_ids=list(range(NCORES)), **kwargs
    )
    LAST_RESULTS = res
    return np.concatenate([r["out"] for r in res.results], axis=0)
